# revision 8
# baseline (speedup 1.0000x reference)
"""Trainium2 Bass kernel v2 for nn_Apply2DDispField (displacement-field
bilinear sampling). Data parallel over batch: 8 cores x 2 images.

Gather strategy (replaces v1's per-column indirect DMAs at ~11 ns/pixel):
  - DRAM block table per image: record (x0, q) = rows {x0, x0+1} x cols
    [32q .. 32q+32] in bf16 (2x33 = 66 values, 132 B) padded to a 256 B
    stride. Record index = x0*32 + q <= 32767 fits dma_gather's int16.
  - nc.gpsimd.dma_gather with 1024 indices/call, rotated over 4 SWDGE
    queues (HW-probed: 7.5 ns/idx on 1 queue, ~0.9 ns/idx on 4; >=2048
    indices/call or mid-stream negative indices hang the ucode).
  - Residual 1-of-34 column select done arithmetically on DVE with
    zero-stride broadcast APs and max-extraction: taps are strictly < 1,
    so max(rec + onehot(j0)) = 1 + tap; four masked max-reduces per pixel
    recover the 2x2 taps, and the bilinear blend (minus the onehot mass
    C = (wx0+wx1)(wy0+wy1)) runs chunk-wide in fp32.
  - Fully-clipped pixels (~64%: displacement ~N(0,1) in [-1,1] coords)
    have all-zero weights; their gather indices are remapped to a
    consecutive per-call pattern to avoid HBM hot-bank serialization on
    the handful of border records.
  - Border clipping is folded into weight masks exactly as v1 (weights
    zero out-of-range taps; padded row/col 1024 are zeros), so the gather
    is a pure clamped fetch and the arithmetic matches the reference
    op-for-op. bf16 table+coefY quantization puts rel err ~6e-3.

Dispatch: cached jax.jit(shard_map(bass_exec)) executable with
device-resident staged inputs and recycled donated output buffers (same
machinery as v1).
"""

import sys

sys.path.insert(0, "/opt/trn_rl_repo")

import numpy as np
from contextlib import ExitStack

import concourse.bass as bass
import concourse.tile as tile
from concourse import bacc, mybir
from concourse.bass import AP
from concourse.library_config import mlp

F32 = mybir.dt.float32
BF16 = mybir.dt.bfloat16
I16 = mybir.dt.int16
Alu = mybir.AluOpType

N_CORES = 8
B_TOTAL = 16
BC = B_TOTAL // N_CORES  # images per core
H = W = 1024
HW = H * W
CH = 128  # pixel rows per chunk (= partitions)
N_CHUNKS = H // CH  # per image
NREC = 32768  # table records per image (1024 x0 * 32 q)
ELEM = 128  # bf16 elements fetched per record (256 B)
NI = 1024  # indices per dma_gather call (hard ucode envelope)
CALLS = W * CH // NI  # gather calls per chunk (128)
SBW = 8  # gather calls per super-block (select granularity: 64 px cols)
N_SB = CALLS // SBW
NQ = 4  # SWDGE queues
MAGIC = float(np.float32(12582912.0))  # 2^23 + 2^22 fp32 round magic
WIN = 34  # select window width (cols per record row, even for 2x DVE)
ROW1 = 34  # element offset of row1 within a record (4B-aligned for 2x DVE)
SKIP_GATHER = False  # profiling: skip the dma_gather calls
SKIP_SELECT = False  # profiling: skip the DVE select ops
SKIP_STAGE = False   # profiling: skip idx staging hops


def _flat_ap(t, offset, pattern):
    """Raw AP over a dram tensor: pattern = [(step, num), ...] in elements."""
    return AP(tensor=t, offset=offset, ap=[list(p) for p in pattern])


def _bc(t_ap, pattern, offset_elems=0):
    """Custom free-dim pattern (incl. zero strides) on an SBUF tile AP."""
    return AP(tensor=t_ap.tensor,
              offset=t_ap.offset + offset_elems,
              ap=[list(t_ap.ap[0])] + [list(p) for p in pattern])


def build_nc(reps: int = 1):
    nc = bacc.Bacc("TRN2", target_bir_lowering=False, debug=False,
                   num_devices=N_CORES, num_swdge_queues=NQ)

    # Img host-padded with one extra zero row per image (H+1 rows)
    img = nc.dram_tensor("Img", [BC, H + 1, W], F32, kind="ExternalInput")
    disp = nc.dram_tensor("Disp", [BC, H, W, 2], F32, kind="ExternalInput")
    rowA = nc.dram_tensor("rowA", [H, 1], F32, kind="ExternalInput")
    colA = nc.dram_tensor("colA", [128, W], F32, kind="ExternalInput")
    iotaS = nc.dram_tensor("iotaS", [128, WIN], BF16, kind="ExternalInput")
    junkA = nc.dram_tensor("junkA", [128, W], F32, kind="ExternalInput")
    out = nc.dram_tensor("out", [BC, H, W], F32, kind="ExternalOutput")

    tb = [nc.dram_tensor(f"tb{b}", [NREC, ELEM], BF16, kind="Internal")
          for b in range(BC)]

    with tile.TileContext(nc) as tc, ExitStack() as top:
        nc.gpsimd.load_library(mlp)

        const_pool = top.enter_context(tc.tile_pool(name="consts", bufs=1))
        ayb = const_pool.tile([128, W], F32)
        nc.sync.dma_start(ayb[:], colA.ap())
        iot = const_pool.tile([128, WIN], BF16)
        nc.sync.dma_start(iot[:], iotaS.ap())
        junk = const_pool.tile([128, W], F32)
        nc.sync.dma_start(junk[:], junkA.ap())

        def build_table(b):
            """Block table for image b: rec[p, x', q, :] covers
            x0 = 8p + x', cols [32q, 32q+32], rows {x0, x0+1}."""
            with ExitStack() as ctx:
                impool = ctx.enter_context(tc.tile_pool(name=f"tbi{b}", bufs=1))
                rpool = ctx.enter_context(tc.tile_pool(name=f"tbr{b}", bufs=1))
                # im[p, :] = Img[b, 8p : 8p+9, :] (9 rows: 1-row overlap;
                # row 1024 is the host-provided zero pad row)
                im = impool.tile([128, 9 * W + 4], F32)
                nc.vector.memset(im[:, 9 * W:], 0.0)
                src = _flat_ap(img, b * (H + 1) * W,
                               [(8 * W, 128), (1, 9 * W)])
                nc.sync.dma_start(im[:, 0:9 * W], src)

                rec = rpool.tile([128, 8 * 32 * ELEM], BF16)
                for r in (0, 1):
                    # cols j in [0,32): rec[x', q, r*34 + j] = im[(x'+r)*W + 32q + j]
                    dst = _bc(rec[:], [(32 * ELEM, 8), (ELEM, 32), (1, 32)],
                              r * ROW1)
                    srcp = _bc(im[:], [(W, 8), (32, 32), (1, 32)], r * W)
                    nc.vector.tensor_copy(dst, srcp)
                    # cols j=32,33 for q < 31: im cols 32q+32, 32q+33
                    dst2 = _bc(rec[:], [(32 * ELEM, 8), (ELEM, 31), (1, 2)],
                               r * ROW1 + 32)
                    src2 = _bc(im[:], [(W, 8), (32, 31), (1, 2)], r * W + 32)
                    nc.vector.tensor_copy(dst2, src2)
                # cols 1024/1025 (q=31, j=32,33): zero pad
                zap = _bc(rec[:], [(32 * ELEM, 8), (ROW1, 2), (1, 2)],
                          31 * ELEM + 32)
                nc.vector.memset(zap, 0.0)
                dst = _flat_ap(tb[b], 0, [(8 * 32 * ELEM, 128),
                                          (1, 8 * 32 * ELEM)])
                nc.sync.dma_start(dst, rec[:])

        def do_chunk(pools, b, t):
            r0 = t * CH
            dpool, cpool, cpool2, wpool, midpool, stpool, spool, opool = pools
            d = dpool.tile([128, W, 2], F32, tag="disp")
            nc.sync.dma_start(d[:], disp.ap()[b, r0:r0 + 128])
            axs = cpool2.tile([128, 1], F32, tag="axs")
            nc.sync.dma_start(axs[:], rowA.ap()[r0:r0 + 128])

            def coord(dcomp, grid_scalar, grid_tensor, xtag, aftag):
                # exact reference op order: xs = ax - d; x = (0.5*(xs+1))*1023
                u = cpool.tile([128, W], F32, tag="u")
                if grid_scalar is not None:
                    nc.vector.tensor_scalar(u[:], dcomp, grid_scalar, -1.0,
                                            Alu.subtract, Alu.mult)
                else:
                    nc.vector.scalar_tensor_tensor(u[:], dcomp, -1.0,
                                                   grid_tensor, Alu.mult,
                                                   Alu.add)
                x = cpool.tile([128, W], F32, tag=xtag)
                nc.vector.tensor_scalar(x[:], u[:], 1.0, 0.5, Alu.add, Alu.mult)
                nc.vector.tensor_scalar_mul(x[:], x[:], float(H - 1))
                af = cpool.tile([128, W], F32, tag=aftag)
                nc.vector.tensor_scalar(af[:], x[:], MAGIC, MAGIC, Alu.add,
                                        Alu.subtract)
                return x, af

            x, afx = coord(d[:, :, 0], axs[:, 0:1], None, "xv", "afx")
            y, afy = coord(d[:, :, 1], None, ayb[:], "yv", "afy")

            def weights(af, xc, w0out, w1out):
                # w0 = (af+1-x)*[0<=af<=1023];  w1 = (x-af)*[0<=af<=1022]
                w0r = cpool.tile([128, W], F32, tag="w0r")
                nc.vector.scalar_tensor_tensor(w0r[:], af, 1.0, xc, Alu.add,
                                               Alu.subtract)
                w1r = cpool.tile([128, W], F32, tag="w1r")
                nc.vector.tensor_tensor(w1r[:], xc, af, Alu.subtract)
                g = cpool.tile([128, W], F32, tag="u")
                nc.vector.tensor_scalar(g[:], af, 0.0, None, Alu.is_ge)
                nc.vector.scalar_tensor_tensor(w0r[:], af, 1023.0, w0r[:],
                                               Alu.is_le, Alu.mult)
                nc.vector.scalar_tensor_tensor(w1r[:], af, 1022.0, w1r[:],
                                               Alu.is_le, Alu.mult)
                nc.vector.tensor_tensor(w0out, w0r[:], g[:], Alu.mult)
                nc.vector.tensor_tensor(w1out, w1r[:], g[:], Alu.mult)

            wx0 = cpool.tile([128, W], F32, tag="wx0")
            wx1 = cpool.tile([128, W], F32, tag="wx1")
            weights(afx[:], x[:], wx0[:], wx1[:])
            wy0b = cpool.tile([128, W], BF16, tag="wy0b")
            wy1b = cpool.tile([128, W], BF16, tag="wy1b")
            weights(afy[:], y[:], wy0b[:], wy1b[:])
            # alive = [0 <= afx <= 1023] * [0 <= afy <= 1023]
            mxy = cpool.tile([128, W], F32, tag="mxy")
            mt = cpool.tile([128, W], F32, tag="u")
            nc.vector.tensor_scalar(mt[:], afx[:], 0.0, None, Alu.is_ge)
            nc.vector.scalar_tensor_tensor(mxy[:], afx[:], 1023.0, mt[:],
                                           Alu.is_le, Alu.mult)
            nc.vector.tensor_scalar(mt[:], afy[:], 0.0, None, Alu.is_ge)
            nc.vector.scalar_tensor_tensor(mt[:], afy[:], 1023.0, mt[:],
                                           Alu.is_le, Alu.mult)
            nc.vector.tensor_tensor(mxy[:], mxy[:], mt[:], Alu.mult)

            # gather index = clamp(afx,0,1023)*32 + (clamp(afy,0,1023) >> 5)
            acl = cpool.tile([128, W], F32, tag="xv")
            nc.vector.tensor_scalar(acl[:], afx[:], 0.0, float(H - 1), Alu.max,
                                    Alu.min)
            ccl = cpool.tile([128, W], F32, tag="yv")
            nc.vector.tensor_scalar(ccl[:], afy[:], 0.0, float(W - 1), Alu.max,
                                    Alu.min)
            # qf = floor(ccl / 32) via round(ccl/32 - 0.499) (magic round)
            qf = cpool.tile([128, W], F32, tag="afx")
            nc.vector.tensor_scalar(qf[:], ccl[:], 0.03125, -0.499, Alu.mult,
                                    Alu.add)
            nc.vector.tensor_scalar(qf[:], qf[:], MAGIC, MAGIC, Alu.add,
                                    Alu.subtract)
            # j0 = ccl - 32*qf  (bf16: integer 0..31, exact)
            j0b = cpool.tile([128, W], BF16, tag="j0b")
            nc.vector.scalar_tensor_tensor(j0b[:], qf[:], -32.0, ccl[:],
                                           Alu.mult, Alu.add)
            j0p1b = cpool.tile([128, W], BF16, tag="j0p1b")
            nc.vector.tensor_scalar(j0p1b[:], j0b[:], 1.0, None, Alu.add)
            idxf = cpool.tile([128, W], F32, tag="w0r")
            nc.vector.scalar_tensor_tensor(idxf[:], acl[:], 32.0, qf[:],
                                           Alu.mult, Alu.add)
            # Fully-clipped pixels have all-zero weights; their gathered
            # value is unused. Spread their indices uniformly over the table
            # (via a host-random pattern) to avoid HBM hot-bank serialization
            # on the handful of border records (~64% of pixels clip).
            t1 = cpool.tile([128, W], F32, tag="w1r")
            nc.vector.tensor_tensor(idxf[:], idxf[:], junk[:], Alu.subtract)
            nc.vector.tensor_tensor(idxf[:], idxf[:], mxy[:], Alu.mult)
            nc.vector.tensor_tensor(idxf[:], idxf[:], junk[:], Alu.add)
            del t1
            idx16 = cpool2.tile([128, W], I16, tag="idx16")
            nc.vector.tensor_copy(idx16[:], idxf[:])

            # Stage indices into dma_gather's wrapped layout (idx of chunk
            # pixel (p, j) -> partition p%16 (replicated over the 8 groups),
            # free slot (j//8)*64 + (j%8)*8 + p//16), all within SBUF:
            #  hop1: fold partition p=16k+q -> partition q, free k*1024+j
            #  hop2: DVE per-partition transpose (cj', k) -> k-inner
            #  hop3: replicate partitions [0:16) to the other 7 groups
            wr = wpool.tile([128, CALLS * (NI // 16)], I16, tag="wr")
            mid = midpool.tile([16, CALLS * (NI // 16)], I16, tag="mid")
            if not SKIP_STAGE:
                for k in range(8):
                    nc.sync.dma_start(mid[:, k * W:(k + 1) * W],
                                      idx16[16 * k:16 * (k + 1), :])
                nc.vector.tensor_copy(
                    _bc(wr[0:16, :], [(8, W), (1, 8)]),
                    _bc(mid[:], [(1, W), (W, 8)]))
                for g in range(1, 8):
                    nc.sync.dma_start(wr[16 * g:16 * (g + 1), :], wr[0:16, :])
            else:
                nc.vector.memset(wr[:, 0:1], 0)

            tb_ap = _flat_ap(tb[b], 0, [(ELEM, NREC), (1, ELEM)])
            ot = opool.tile([128, W], F32, tag="ot")
            M00 = spool.tile([128, W], BF16, tag="M00")
            M01 = spool.tile([128, W], BF16, tag="M01")
            M10 = spool.tile([128, W], BF16, tag="M10")
            M11 = spool.tile([128, W], BF16, tag="M11")
            for cs in range(N_SB):
                st = stpool.tile([128, SBW * 8, ELEM], BF16, tag="st")
                if not SKIP_GATHER:
                    for cc in range(SBW):
                        c = cs * SBW + cc
                        nc.gpsimd.dma_gather(
                            st[:, 8 * cc:8 * cc + 8, :], tb_ap,
                            wr[:, 64 * c:64 * c + 64], NI, NI, ELEM,
                            queue_num=c % NQ)
                else:
                    nc.vector.memset(st[:, 0:1, 0:2], 0.25)
                # select v2 (max-extraction): taps are strictly < 1, so
                # max(rec + onehot) = 1 + tap; subtract the onehot mass via
                # C = (wx0+wx1)*(wy0+wy1) in the chunk-level blend.
                sl = slice(64 * cs, 64 * cs + 64)
                npx = 64
                if SKIP_SELECT:
                    for Mt in (M00, M01, M10, M11):
                        nc.vector.memset(Mt[:, sl], 1.5)
                    continue
                ohA = spool.tile([128, npx, WIN], BF16, tag="ohA")
                nc.vector.tensor_tensor(
                    ohA[:],
                    _bc(j0b[:, sl], [(1, npx), (0, WIN)]),
                    _bc(iot[:], [(0, npx), (1, WIN)]),
                    Alu.is_equal)
                ohB = spool.tile([128, npx, WIN], BF16, tag="ohB")
                nc.vector.tensor_tensor(
                    ohB[:],
                    _bc(j0p1b[:, sl], [(1, npx), (0, WIN)]),
                    _bc(iot[:], [(0, npx), (1, WIN)]),
                    Alu.is_equal)
                m = spool.tile([128, npx, WIN], BF16, tag="m")
                for Mt, row_off, oh in ((M00, 0, ohA), (M01, 0, ohB),
                                        (M10, ROW1, ohA), (M11, ROW1, ohB)):
                    nc.vector.tensor_tensor(
                        m[:], st[:, :, row_off:row_off + WIN], oh[:], Alu.add)
                    nc.vector.tensor_reduce(Mt[:, sl], m[:],
                                            mybir.AxisListType.X, Alu.max)
            # chunk-level blend on [128, W]:
            # out = wx0*(wy0*M00 + wy1*M01) + wx1*(wy0*M10 + wy1*M11) - C
            av = cpool.tile([128, W], F32, tag="w0r")
            bv = cpool.tile([128, W], F32, tag="w1r")
            cv = cpool.tile([128, W], F32, tag="u")
            nc.vector.tensor_tensor(av[:], M00[:], wy0b[:], Alu.mult)
            nc.vector.tensor_tensor(bv[:], M01[:], wy1b[:], Alu.mult)
            nc.vector.tensor_tensor(av[:], av[:], bv[:], Alu.add)
            nc.vector.tensor_tensor(bv[:], M10[:], wy0b[:], Alu.mult)
            nc.vector.tensor_tensor(cv[:], M11[:], wy1b[:], Alu.mult)
            nc.vector.tensor_tensor(bv[:], bv[:], cv[:], Alu.add)
            nc.vector.tensor_tensor(av[:], av[:], wx0[:], Alu.mult)
            nc.vector.tensor_tensor(bv[:], bv[:], wx1[:], Alu.mult)
            nc.vector.tensor_tensor(av[:], av[:], bv[:], Alu.add)
            # C = (wx0+wx1)*(wy0+wy1)
            nc.vector.tensor_tensor(bv[:], wx0[:], wx1[:], Alu.add)
            nc.vector.tensor_tensor(cv[:], wy0b[:], wy1b[:], Alu.add)
            nc.vector.tensor_tensor(bv[:], bv[:], cv[:], Alu.mult)
            nc.vector.tensor_tensor(ot[:], av[:], bv[:], Alu.subtract)
            nc.sync.dma_start(out.ap()[b, r0:r0 + 128], ot[:])

        def body(iv=None):
            for b in range(BC):
                build_table(b)
            with ExitStack() as cctx:
                dpool = cctx.enter_context(tc.tile_pool(name="dpool", bufs=2))
                cpool = cctx.enter_context(tc.tile_pool(name="cpool", bufs=1))
                cpool2 = cctx.enter_context(tc.tile_pool(name="cpool2", bufs=2))
                wpool = cctx.enter_context(tc.tile_pool(name="wpool", bufs=2))
                midpool = cctx.enter_context(tc.tile_pool(name="midpool", bufs=1))
                stpool = cctx.enter_context(tc.tile_pool(name="stpool", bufs=3))
                spool = cctx.enter_context(tc.tile_pool(name="spool", bufs=1))
                opool = cctx.enter_context(tc.tile_pool(name="opool", bufs=2))
                pools = (dpool, cpool, cpool2, wpool, midpool, stpool, spool,
                         opool)
                for b in range(BC):
                    for t in range(N_CHUNKS):
                        do_chunk(pools, b, t)

        if reps == 1:
            body()
        else:
            with tc.For_i(0, reps, 1) as i:
                body(i)

    nc.compile()
    return nc


_CACHED = {}


def _get_nc(reps=1):
    if reps not in _CACHED:
        _CACHED[reps] = build_nc(reps)
    return _CACHED[reps]


def make_in_maps(Img: np.ndarray, DispField: np.ndarray):
    import ml_dtypes
    Img = np.asarray(Img, dtype=np.float32).reshape(B_TOTAL, H, W)
    Img = np.ascontiguousarray(
        np.pad(Img, ((0, 0), (0, 1), (0, 0))))  # zero row H per image
    Disp = np.ascontiguousarray(
        np.asarray(DispField, dtype=np.float32).reshape(B_TOTAL, H, W, 2))
    grid = np.linspace(-1.0, 1.0, H).astype(np.float32)
    rowA = np.ascontiguousarray(grid.reshape(H, 1))
    colA = np.ascontiguousarray(np.broadcast_to(grid, (128, W)))
    iotaS = np.ascontiguousarray(
        np.broadcast_to(np.arange(WIN, dtype=np.float32),
                        (128, WIN)).astype(ml_dtypes.bfloat16))
    # consecutive per (column-block) call: dead-pixel reads walk contiguous
    # records (HBM row-buffer hits) instead of random ones
    jj = (np.arange(W)[None, :] * 128 + np.arange(128)[:, None]) % NREC
    junkA = jj.astype(np.float32)
    in_maps = []
    for c in range(N_CORES):
        in_maps.append({
            "Img": Img[c * BC:(c + 1) * BC],
            "Disp": Disp[c * BC:(c + 1) * BC],
            "rowA": rowA,
            "colA": colA,
            "iotaS": iotaS,
            "junkA": junkA,
        })
    return in_maps


# ---------------------------------------------------------------------------
# Fast dispatch: cached jitted executable + device-resident inputs.
#
# bass_utils.run_bass_kernel_spmd under axon rebuilds the jax.jit(shard_map)
# callable and re-uploads every input array on every call. We mirror its
# lowering exactly (bass2jax.run_bass_via_pjrt) but hoist the jit and the
# input staging out of the per-run path, and recycle the previous run's
# output buffers as this run's donated output buffers (the kernel writes
# every element of "out", so their contents don't matter).
# ---------------------------------------------------------------------------

_EXEC_CACHE = {}
_STAGE_CACHE = {}


def _get_executor(nc):
    key = id(nc)
    if key in _EXEC_CACHE:
        return _EXEC_CACHE[key]

    import jax
    import jax.numpy as jnp
    from concourse import bass2jax
    from jax.sharding import Mesh, PartitionSpec, NamedSharding
    try:
        from jax.experimental.shard_map import shard_map
    except ImportError:  # newer jax
        from jax.sharding import shard_map

    bass2jax.install_neuronx_cc_hook()

    assert nc.dbg_addr is None, "build with debug=False"
    partition_name = (nc.partition_id_tensor.name
                      if nc.partition_id_tensor else None)

    in_names = []
    out_names = []
    out_avals = []
    for alloc in nc.m.functions[0].allocations:
        if not isinstance(alloc, mybir.MemoryLocationSet):
            continue
        assert alloc.memorylocations
        name = alloc.memorylocations[0].name
        if alloc.kind == "ExternalInput":
            if name != partition_name:
                in_names.append(name)
        elif alloc.kind == "ExternalOutput":
            assert alloc.tensor_shape is not None and alloc.dtype is not None
            out_names.append(name)
            shape = tuple(alloc.tensor_shape)
            dtype = mybir.dt.np(alloc.dtype)
            out_avals.append(jax.core.ShapedArray(shape, dtype))
    n_params = len(in_names)
    n_outs = len(out_avals)
    all_names = list(in_names) + list(out_names)
    if partition_name is not None:
        all_names.append(partition_name)
    donate = tuple(range(n_params, n_params + n_outs))

    def _body(*args):
        operands = list(args)
        if partition_name is not None:
            operands.append(bass2jax.partition_id_tensor())
        outs = bass2jax._bass_exec_p.bind(
            *operands,
            out_avals=tuple(out_avals),
            in_names=tuple(all_names),
            out_names=tuple(out_names),
            lowering_input_output_aliases=(),
            sim_require_finite=True,
            sim_require_nnan=True,
            nc=nc,
        )
        return tuple(outs)

    devices = jax.devices()[:N_CORES]
    assert len(devices) == N_CORES
    mesh = Mesh(np.asarray(devices), ("core",))
    pspec = PartitionSpec("core")
    sharding = NamedSharding(mesh, pspec)
    in_specs = (pspec,) * (n_params + n_outs)
    out_specs = (pspec,) * n_outs
    fn = jax.jit(
        shard_map(_body, mesh=mesh, in_specs=in_specs, out_specs=out_specs,
                  check_rep=False),
        donate_argnums=donate,
        keep_unused=True,
    )

    glob_out_shapes = [(N_CORES * a.shape[0],) + tuple(a.shape[1:])
                      for a in out_avals]
    glob_out_dtypes = [a.dtype for a in out_avals]

    def make_zeros():
        mk = jax.jit(
            lambda: tuple(jnp.zeros(s, d) for s, d in
                          zip(glob_out_shapes, glob_out_dtypes)),
            out_shardings=tuple(sharding for _ in glob_out_shapes),
        )
        return list(mk())

    ex = {
        "jax": jax,
        "fn": fn,
        "in_names": in_names,
        "out_names": out_names,
        "out_avals": out_avals,
        "sharding": sharding,
        "make_zeros": make_zeros,
    }
    _EXEC_CACHE[key] = ex
    return ex


def _stage(ex, in_maps):
    key = id(in_maps)
    hit = _STAGE_CACHE.get(key)
    if hit is not None and hit["pin"] is in_maps:
        return hit
    jax = ex["jax"]
    concat = [
        np.concatenate([np.asarray(m[name]) for m in in_maps], axis=0)
        for name in ex["in_names"]
    ]
    dev_inputs = [jax.device_put(a, ex["sharding"]) for a in concat]
    jax.block_until_ready(dev_inputs)
    staged = {
        "pin": in_maps,  # strong ref keeps id() stable
        "dev_inputs": dev_inputs,
        "donation": ex["make_zeros"](),
    }
    if len(_STAGE_CACHE) >= 2:  # bound device/host memory pinned by the cache
        _STAGE_CACHE.pop(next(iter(_STAGE_CACHE)))
    _STAGE_CACHE[key] = staged
    return staged


class _RunResult:
    """Per-run device outputs; host transfer happens lazily, once."""

    def __init__(self, ex, outs):
        self._ex = ex
        self._outs = outs
        self._host = None

    def host(self, name):
        if self._host is None:
            self._host = {
                n: np.asarray(a)
                for n, a in zip(self._ex["out_names"], self._outs)
            }
        return self._host[name]


class _CoreView:
    """numpy-convertible view of one core's slice of a global output."""

    def __init__(self, runres, name, core, core_shape):
        self._runres = runres
        self._name = name
        self._core = core
        self._core_shape = core_shape

    def __array__(self, dtype=None, copy=None):
        full = self._runres.host(self._name)
        arr = full.reshape((N_CORES,) + self._core_shape)[self._core]
        if dtype is not None:
            arr = arr.astype(dtype)
        return arr


def _execute(ex, staged):
    jax = ex["jax"]
    outs = list(ex["fn"](*staged["dev_inputs"], *staged["donation"]))
    jax.block_until_ready(outs)
    # recycle: this run's outputs become next run's donated buffers
    staged["donation"] = outs
    return _RunResult(ex, outs)




def run_on_cores(in_maps, reps=1):
    nc = _get_nc(reps)
    try:
        ex = _get_executor(nc)
        staged = _stage(ex, in_maps)
        rr = _execute(ex, staged)
        res = []
        for c in range(N_CORES):
            res.append({
                name: _CoreView(rr, name, c, tuple(aval.shape))
                for name, aval in zip(ex["out_names"], ex["out_avals"])
            })
        return res
    except Exception:
        from concourse.bass_utils import run_bass_kernel_spmd
        res = run_bass_kernel_spmd(nc, in_maps, core_ids=list(range(N_CORES)),
                                   trace=False)
        return res.results if hasattr(res, "results") else res


def kernel(Img: np.ndarray, DispField: np.ndarray) -> np.ndarray:
    in_maps = make_in_maps(Img, DispField)
    results = run_on_cores(in_maps)
    out = np.concatenate([np.asarray(r["out"]) for r in results], axis=0)
    return out.reshape(B_TOTAL, H, W, 1).astype(np.float32)


if __name__ == "__main__":
    rng = np.random.default_rng(0)
    Img = rng.random((B_TOTAL, H, W, 1), dtype=np.float32)
    Disp = rng.standard_normal((B_TOTAL, H, W, 2)).astype(np.float32)
    o = kernel(Img, Disp)
    print("out", o.shape, o.dtype, float(np.abs(o).mean()))


# revision 10
# speedup vs baseline: 1.0017x; 1.0017x over previous
"""Trainium2 Bass kernel v2 for nn_Apply2DDispField (displacement-field
bilinear sampling). Data parallel over batch: 8 cores x 2 images.

Gather strategy (replaces v1's per-column indirect DMAs at ~11 ns/pixel):
  - DRAM block table per image: record (x0, q) = rows {x0, x0+1} x cols
    [32q .. 32q+32] in bf16 (2x33 = 66 values, 132 B) padded to a 256 B
    stride. Record index = x0*32 + q <= 32767 fits dma_gather's int16.
  - nc.gpsimd.dma_gather with 1024 indices/call, rotated over 4 SWDGE
    queues (HW-probed: 7.5 ns/idx on 1 queue, ~0.9 ns/idx on 4; >=2048
    indices/call or mid-stream negative indices hang the ucode).
  - Residual 1-of-34 column select done arithmetically on DVE with
    zero-stride broadcast APs and max-extraction: taps are strictly < 1,
    so max(rec + onehot(j0)) = 1 + tap; four masked max-reduces per pixel
    recover the 2x2 taps, and the bilinear blend (minus the onehot mass
    C = (wx0+wx1)(wy0+wy1)) runs chunk-wide in fp32.
  - Fully-clipped pixels (~64%: displacement ~N(0,1) in [-1,1] coords)
    have all-zero weights; their gather indices are remapped to a
    consecutive per-call pattern to avoid HBM hot-bank serialization on
    the handful of border records.
  - Border clipping is folded into weight masks exactly as v1 (weights
    zero out-of-range taps; padded row/col 1024 are zeros), so the gather
    is a pure clamped fetch and the arithmetic matches the reference
    op-for-op. bf16 table+coefY quantization puts rel err ~6e-3.

Dispatch: cached jax.jit(shard_map(bass_exec)) executable with
device-resident staged inputs and recycled donated output buffers (same
machinery as v1).
"""

import sys

sys.path.insert(0, "/opt/trn_rl_repo")

import numpy as np
from contextlib import ExitStack

import concourse.bass as bass
import concourse.tile as tile
from concourse import bacc, mybir
from concourse.bass import AP
from concourse.library_config import mlp

F32 = mybir.dt.float32
BF16 = mybir.dt.bfloat16
I16 = mybir.dt.int16
Alu = mybir.AluOpType

N_CORES = 8
B_TOTAL = 16
BC = B_TOTAL // N_CORES  # images per core
H = W = 1024
HW = H * W
CH = 128  # pixel rows per chunk (= partitions)
N_CHUNKS = H // CH  # per image
NREC = 32768  # table records per image (1024 x0 * 32 q)
ELEM = 128  # bf16 elements fetched per record (256 B)
NI = 1024  # indices per dma_gather call (hard ucode envelope)
CALLS = W * CH // NI  # gather calls per chunk (128)
SBW = 8  # gather calls per super-block (select granularity: 64 px cols)
N_SB = CALLS // SBW
NQ = 4  # SWDGE queues
MAGIC = float(np.float32(12582912.0))  # 2^23 + 2^22 fp32 round magic
WIN = 34  # select window width (cols per record row, even for 2x DVE)
ROW1 = 34  # element offset of row1 within a record (4B-aligned for 2x DVE)
SKIP_GATHER = False  # profiling: skip the dma_gather calls
SKIP_SELECT = False  # profiling: skip the DVE select ops
SKIP_STAGE = False   # profiling: skip idx staging hops


def _flat_ap(t, offset, pattern):
    """Raw AP over a dram tensor: pattern = [(step, num), ...] in elements."""
    return AP(tensor=t, offset=offset, ap=[list(p) for p in pattern])


def _bc(t_ap, pattern, offset_elems=0):
    """Custom free-dim pattern (incl. zero strides) on an SBUF tile AP."""
    return AP(tensor=t_ap.tensor,
              offset=t_ap.offset + offset_elems,
              ap=[list(t_ap.ap[0])] + [list(p) for p in pattern])


def build_nc(reps: int = 1):
    nc = bacc.Bacc("TRN2", target_bir_lowering=False, debug=False,
                   num_devices=N_CORES, num_swdge_queues=NQ)

    # Img host-padded with one extra zero row per image (H+1 rows)
    img = nc.dram_tensor("Img", [BC, H + 1, W], F32, kind="ExternalInput")
    disp = nc.dram_tensor("Disp", [BC, H, W, 2], F32, kind="ExternalInput")
    rowA = nc.dram_tensor("rowA", [H, 1], F32, kind="ExternalInput")
    colA = nc.dram_tensor("colA", [128, W], F32, kind="ExternalInput")
    iotaS = nc.dram_tensor("iotaS", [128, WIN], BF16, kind="ExternalInput")
    junkA = nc.dram_tensor("junkA", [128, W], F32, kind="ExternalInput")
    out = nc.dram_tensor("out", [BC, H, W], F32, kind="ExternalOutput")

    tb = [nc.dram_tensor(f"tb{b}", [NREC, ELEM], BF16, kind="Internal")
          for b in range(BC)]

    with tile.TileContext(nc) as tc, ExitStack() as top:
        nc.gpsimd.load_library(mlp)

        const_pool = top.enter_context(tc.tile_pool(name="consts", bufs=1))
        ayb = const_pool.tile([128, W], F32)
        nc.sync.dma_start(ayb[:], colA.ap())
        iot = const_pool.tile([128, WIN], BF16)
        nc.sync.dma_start(iot[:], iotaS.ap())
        junk = const_pool.tile([128, W], F32)
        nc.sync.dma_start(junk[:], junkA.ap())

        def build_table(b):
            """Block table for image b: rec[p, x', q, :] covers
            x0 = 8p + x', cols [32q, 32q+32], rows {x0, x0+1}."""
            with ExitStack() as ctx:
                impool = ctx.enter_context(tc.tile_pool(name=f"tbi{b}", bufs=1))
                rpool = ctx.enter_context(tc.tile_pool(name=f"tbr{b}", bufs=1))
                # im[p, :] = Img[b, 8p : 8p+9, :] (9 rows: 1-row overlap;
                # row 1024 is the host-provided zero pad row)
                im = impool.tile([128, 9 * W + 4], F32)
                nc.vector.memset(im[:, 9 * W:], 0.0)
                src = _flat_ap(img, b * (H + 1) * W,
                               [(8 * W, 128), (1, 9 * W)])
                nc.sync.dma_start(im[:, 0:9 * W], src)

                rec = rpool.tile([128, 8 * 32 * ELEM], BF16)
                for r in (0, 1):
                    # cols j in [0,32): rec[x', q, r*34 + j] = im[(x'+r)*W + 32q + j]
                    dst = _bc(rec[:], [(32 * ELEM, 8), (ELEM, 32), (1, 32)],
                              r * ROW1)
                    srcp = _bc(im[:], [(W, 8), (32, 32), (1, 32)], r * W)
                    nc.vector.tensor_copy(dst, srcp)
                    # cols j=32,33 for q < 31: im cols 32q+32, 32q+33
                    dst2 = _bc(rec[:], [(32 * ELEM, 8), (ELEM, 31), (1, 2)],
                               r * ROW1 + 32)
                    src2 = _bc(im[:], [(W, 8), (32, 31), (1, 2)], r * W + 32)
                    nc.vector.tensor_copy(dst2, src2)
                # cols 1024/1025 (q=31, j=32,33): zero pad
                zap = _bc(rec[:], [(32 * ELEM, 8), (ROW1, 2), (1, 2)],
                          31 * ELEM + 32)
                nc.vector.memset(zap, 0.0)
                dst = _flat_ap(tb[b], 0, [(8 * 32 * ELEM, 128),
                                          (1, 8 * 32 * ELEM)])
                nc.sync.dma_start(dst, rec[:])

        def do_chunk(pools, b, t):
            r0 = t * CH
            dpool, cpool, cpool2, wpool, midpool, stpool, spool, opool = pools
            d = dpool.tile([128, W, 2], F32, tag="disp")
            nc.sync.dma_start(d[:], disp.ap()[b, r0:r0 + 128])
            axs = cpool2.tile([128, 1], F32, tag="axs")
            nc.sync.dma_start(axs[:], rowA.ap()[r0:r0 + 128])

            def coord(dcomp, grid_scalar, grid_tensor, xtag, aftag):
                # exact reference op order: xs = ax - d; x = (0.5*(xs+1))*1023
                u = cpool.tile([128, W], F32, tag="u")
                if grid_scalar is not None:
                    nc.vector.tensor_scalar(u[:], dcomp, grid_scalar, -1.0,
                                            Alu.subtract, Alu.mult)
                else:
                    nc.vector.scalar_tensor_tensor(u[:], dcomp, -1.0,
                                                   grid_tensor, Alu.mult,
                                                   Alu.add)
                x = cpool.tile([128, W], F32, tag=xtag)
                nc.vector.tensor_scalar(x[:], u[:], 1.0, 0.5, Alu.add, Alu.mult)
                nc.vector.tensor_scalar_mul(x[:], x[:], float(H - 1))
                af = cpool.tile([128, W], F32, tag=aftag)
                nc.vector.tensor_scalar(af[:], x[:], MAGIC, MAGIC, Alu.add,
                                        Alu.subtract)
                return x, af

            x, afx = coord(d[:, :, 0], axs[:, 0:1], None, "xv", "afx")
            y, afy = coord(d[:, :, 1], None, ayb[:], "yv", "afy")

            def weights(af, xc, w0out, w1out):
                # w0 = (af+1-x)*[0<=af<=1023];  w1 = (x-af)*[0<=af<=1022]
                w0r = cpool.tile([128, W], F32, tag="w0r")
                nc.vector.scalar_tensor_tensor(w0r[:], af, 1.0, xc, Alu.add,
                                               Alu.subtract)
                w1r = cpool.tile([128, W], F32, tag="w1r")
                nc.vector.tensor_tensor(w1r[:], xc, af, Alu.subtract)
                g = cpool.tile([128, W], F32, tag="u")
                nc.vector.tensor_scalar(g[:], af, 0.0, None, Alu.is_ge)
                nc.vector.scalar_tensor_tensor(w0r[:], af, 1023.0, w0r[:],
                                               Alu.is_le, Alu.mult)
                nc.vector.scalar_tensor_tensor(w1r[:], af, 1022.0, w1r[:],
                                               Alu.is_le, Alu.mult)
                nc.vector.tensor_tensor(w0out, w0r[:], g[:], Alu.mult)
                nc.vector.tensor_tensor(w1out, w1r[:], g[:], Alu.mult)

            wx0 = cpool.tile([128, W], F32, tag="wx0")
            wx1 = cpool.tile([128, W], F32, tag="wx1")
            weights(afx[:], x[:], wx0[:], wx1[:])
            wy0b = cpool.tile([128, W], BF16, tag="wy0b")
            wy1b = cpool.tile([128, W], BF16, tag="wy1b")
            weights(afy[:], y[:], wy0b[:], wy1b[:])
            # alive = [0 <= afx <= 1023] * [0 <= afy <= 1023]
            mxy = cpool.tile([128, W], F32, tag="mxy")
            mt = cpool.tile([128, W], F32, tag="u")
            nc.vector.tensor_scalar(mt[:], afx[:], 0.0, None, Alu.is_ge)
            nc.vector.scalar_tensor_tensor(mxy[:], afx[:], 1023.0, mt[:],
                                           Alu.is_le, Alu.mult)
            nc.vector.tensor_scalar(mt[:], afy[:], 0.0, None, Alu.is_ge)
            nc.vector.scalar_tensor_tensor(mt[:], afy[:], 1023.0, mt[:],
                                           Alu.is_le, Alu.mult)
            nc.vector.tensor_tensor(mxy[:], mxy[:], mt[:], Alu.mult)

            # gather index = clamp(afx,0,1023)*32 + (clamp(afy,0,1023) >> 5)
            acl = cpool.tile([128, W], F32, tag="xv")
            nc.vector.tensor_scalar(acl[:], afx[:], 0.0, float(H - 1), Alu.max,
                                    Alu.min)
            ccl = cpool.tile([128, W], F32, tag="yv")
            nc.vector.tensor_scalar(ccl[:], afy[:], 0.0, float(W - 1), Alu.max,
                                    Alu.min)
            # qf = floor(ccl / 32) via round(ccl/32 - 0.499) (magic round)
            qf = cpool.tile([128, W], F32, tag="afx")
            nc.vector.tensor_scalar(qf[:], ccl[:], 0.03125, -0.499, Alu.mult,
                                    Alu.add)
            nc.vector.tensor_scalar(qf[:], qf[:], MAGIC, MAGIC, Alu.add,
                                    Alu.subtract)
            # j0 = ccl - 32*qf  (bf16: integer 0..31, exact)
            j0b = cpool.tile([128, W], BF16, tag="j0b")
            nc.vector.scalar_tensor_tensor(j0b[:], qf[:], -32.0, ccl[:],
                                           Alu.mult, Alu.add)
            j0p1b = cpool.tile([128, W], BF16, tag="j0p1b")
            nc.vector.tensor_scalar(j0p1b[:], j0b[:], 1.0, None, Alu.add)
            idxf = cpool.tile([128, W], F32, tag="w0r")
            nc.vector.scalar_tensor_tensor(idxf[:], acl[:], 32.0, qf[:],
                                           Alu.mult, Alu.add)
            # Fully-clipped pixels have all-zero weights; their gathered
            # value is unused. Spread their indices uniformly over the table
            # (via a host-random pattern) to avoid HBM hot-bank serialization
            # on the handful of border records (~64% of pixels clip).
            t1 = cpool.tile([128, W], F32, tag="w1r")
            nc.vector.tensor_tensor(idxf[:], idxf[:], junk[:], Alu.subtract)
            nc.vector.tensor_tensor(idxf[:], idxf[:], mxy[:], Alu.mult)
            nc.vector.tensor_tensor(idxf[:], idxf[:], junk[:], Alu.add)
            del t1
            idx16 = cpool2.tile([128, W], I16, tag="idx16")
            nc.vector.tensor_copy(idx16[:], idxf[:])

            # Stage indices into dma_gather's wrapped layout (idx of chunk
            # pixel (p, j) -> partition p%16 (replicated over the 8 groups),
            # free slot (j//8)*64 + (j%8)*8 + p//16), all within SBUF:
            #  hop1: fold partition p=16k+q -> partition q, free k*1024+j
            #  hop2: DVE per-partition transpose (cj', k) -> k-inner
            #  hop3: replicate partitions [0:16) to the other 7 groups
            wr = wpool.tile([128, CALLS * (NI // 16)], I16, tag="wr")
            mid = midpool.tile([16, CALLS * (NI // 16)], I16, tag="mid")
            if not SKIP_STAGE:
                for k in range(8):
                    nc.sync.dma_start(mid[:, k * W:(k + 1) * W],
                                      idx16[16 * k:16 * (k + 1), :])
                nc.vector.tensor_copy(
                    _bc(wr[0:16, :], [(8, W), (1, 8)]),
                    _bc(mid[:], [(1, W), (W, 8)]))
                for g in range(1, 8):
                    nc.sync.dma_start(wr[16 * g:16 * (g + 1), :], wr[0:16, :])
            else:
                nc.vector.memset(wr[:, 0:1], 0)

            tb_ap = _flat_ap(tb[b], 0, [(ELEM, NREC), (1, ELEM)])
            ot = opool.tile([128, W], F32, tag="ot")
            M00 = spool.tile([128, W], BF16, tag="M00")
            M01 = spool.tile([128, W], BF16, tag="M01")
            M10 = spool.tile([128, W], BF16, tag="M10")
            M11 = spool.tile([128, W], BF16, tag="M11")
            for cs in range(N_SB):
                st = stpool.tile([128, SBW * 8, ELEM], BF16, tag="st")
                if not SKIP_GATHER:
                    for cc in range(SBW):
                        c = cs * SBW + cc
                        nc.gpsimd.dma_gather(
                            st[:, 8 * cc:8 * cc + 8, :], tb_ap,
                            wr[:, 64 * c:64 * c + 64], NI, NI, ELEM,
                            queue_num=c % NQ, single_packet=False)
                else:
                    nc.vector.memset(st[:, 0:1, 0:2], 0.25)
                # select v2 (max-extraction): taps are strictly < 1, so
                # max(rec + onehot) = 1 + tap; subtract the onehot mass via
                # C = (wx0+wx1)*(wy0+wy1) in the chunk-level blend.
                sl = slice(64 * cs, 64 * cs + 64)
                npx = 64
                if SKIP_SELECT:
                    for Mt in (M00, M01, M10, M11):
                        nc.vector.memset(Mt[:, sl], 1.5)
                    continue
                ohA = spool.tile([128, npx, WIN], BF16, tag="ohA")
                nc.vector.tensor_tensor(
                    ohA[:],
                    _bc(j0b[:, sl], [(1, npx), (0, WIN)]),
                    _bc(iot[:], [(0, npx), (1, WIN)]),
                    Alu.is_equal)
                ohB = spool.tile([128, npx, WIN], BF16, tag="ohB")
                nc.vector.tensor_tensor(
                    ohB[:],
                    _bc(j0p1b[:, sl], [(1, npx), (0, WIN)]),
                    _bc(iot[:], [(0, npx), (1, WIN)]),
                    Alu.is_equal)
                m = spool.tile([128, npx, WIN], BF16, tag="m")
                for Mt, row_off, oh in ((M00, 0, ohA), (M01, 0, ohB),
                                        (M10, ROW1, ohA), (M11, ROW1, ohB)):
                    nc.vector.tensor_tensor(
                        m[:], st[:, :, row_off:row_off + WIN], oh[:], Alu.add)
                    nc.vector.tensor_reduce(Mt[:, sl], m[:],
                                            mybir.AxisListType.X, Alu.max)
            # chunk-level blend on [128, W]:
            # out = wx0*(wy0*M00 + wy1*M01) + wx1*(wy0*M10 + wy1*M11) - C
            av = cpool.tile([128, W], F32, tag="w0r")
            bv = cpool.tile([128, W], F32, tag="w1r")
            cv = cpool.tile([128, W], F32, tag="u")
            nc.vector.tensor_tensor(av[:], M00[:], wy0b[:], Alu.mult)
            nc.vector.tensor_tensor(bv[:], M01[:], wy1b[:], Alu.mult)
            nc.vector.tensor_tensor(av[:], av[:], bv[:], Alu.add)
            nc.vector.tensor_tensor(bv[:], M10[:], wy0b[:], Alu.mult)
            nc.vector.tensor_tensor(cv[:], M11[:], wy1b[:], Alu.mult)
            nc.vector.tensor_tensor(bv[:], bv[:], cv[:], Alu.add)
            nc.vector.tensor_tensor(av[:], av[:], wx0[:], Alu.mult)
            nc.vector.tensor_tensor(bv[:], bv[:], wx1[:], Alu.mult)
            nc.vector.tensor_tensor(av[:], av[:], bv[:], Alu.add)
            # C = (wx0+wx1)*(wy0+wy1)
            nc.vector.tensor_tensor(bv[:], wx0[:], wx1[:], Alu.add)
            nc.vector.tensor_tensor(cv[:], wy0b[:], wy1b[:], Alu.add)
            nc.vector.tensor_tensor(bv[:], bv[:], cv[:], Alu.mult)
            nc.vector.tensor_tensor(ot[:], av[:], bv[:], Alu.subtract)
            nc.sync.dma_start(out.ap()[b, r0:r0 + 128], ot[:])

        def body(iv=None):
            for b in range(BC):
                build_table(b)
            with ExitStack() as cctx:
                dpool = cctx.enter_context(tc.tile_pool(name="dpool", bufs=2))
                cpool = cctx.enter_context(tc.tile_pool(name="cpool", bufs=1))
                cpool2 = cctx.enter_context(tc.tile_pool(name="cpool2", bufs=2))
                wpool = cctx.enter_context(tc.tile_pool(name="wpool", bufs=2))
                midpool = cctx.enter_context(tc.tile_pool(name="midpool", bufs=1))
                stpool = cctx.enter_context(tc.tile_pool(name="stpool", bufs=3))
                spool = cctx.enter_context(tc.tile_pool(name="spool", bufs=1))
                opool = cctx.enter_context(tc.tile_pool(name="opool", bufs=2))
                pools = (dpool, cpool, cpool2, wpool, midpool, stpool, spool,
                         opool)
                for b in range(BC):
                    for t in range(N_CHUNKS):
                        do_chunk(pools, b, t)

        if reps == 1:
            body()
        else:
            with tc.For_i(0, reps, 1) as i:
                body(i)

    nc.compile()
    return nc


_CACHED = {}


def _get_nc(reps=1):
    if reps not in _CACHED:
        _CACHED[reps] = build_nc(reps)
    return _CACHED[reps]


def make_in_maps(Img: np.ndarray, DispField: np.ndarray):
    import ml_dtypes
    Img = np.asarray(Img, dtype=np.float32).reshape(B_TOTAL, H, W)
    Img = np.ascontiguousarray(
        np.pad(Img, ((0, 0), (0, 1), (0, 0))))  # zero row H per image
    Disp = np.ascontiguousarray(
        np.asarray(DispField, dtype=np.float32).reshape(B_TOTAL, H, W, 2))
    grid = np.linspace(-1.0, 1.0, H).astype(np.float32)
    rowA = np.ascontiguousarray(grid.reshape(H, 1))
    colA = np.ascontiguousarray(np.broadcast_to(grid, (128, W)))
    iotaS = np.ascontiguousarray(
        np.broadcast_to(np.arange(WIN, dtype=np.float32),
                        (128, WIN)).astype(ml_dtypes.bfloat16))
    # consecutive per (column-block) call: dead-pixel reads walk contiguous
    # records (HBM row-buffer hits) instead of random ones
    jj = (np.arange(W)[None, :] * 128 + np.arange(128)[:, None]) % NREC
    junkA = jj.astype(np.float32)
    in_maps = []
    for c in range(N_CORES):
        in_maps.append({
            "Img": Img[c * BC:(c + 1) * BC],
            "Disp": Disp[c * BC:(c + 1) * BC],
            "rowA": rowA,
            "colA": colA,
            "iotaS": iotaS,
            "junkA": junkA,
        })
    return in_maps


# ---------------------------------------------------------------------------
# Fast dispatch: cached jitted executable + device-resident inputs.
#
# bass_utils.run_bass_kernel_spmd under axon rebuilds the jax.jit(shard_map)
# callable and re-uploads every input array on every call. We mirror its
# lowering exactly (bass2jax.run_bass_via_pjrt) but hoist the jit and the
# input staging out of the per-run path, and recycle the previous run's
# output buffers as this run's donated output buffers (the kernel writes
# every element of "out", so their contents don't matter).
# ---------------------------------------------------------------------------

_EXEC_CACHE = {}
_STAGE_CACHE = {}


def _get_executor(nc):
    key = id(nc)
    if key in _EXEC_CACHE:
        return _EXEC_CACHE[key]

    import jax
    import jax.numpy as jnp
    from concourse import bass2jax
    from jax.sharding import Mesh, PartitionSpec, NamedSharding
    try:
        from jax.experimental.shard_map import shard_map
    except ImportError:  # newer jax
        from jax.sharding import shard_map

    bass2jax.install_neuronx_cc_hook()

    assert nc.dbg_addr is None, "build with debug=False"
    partition_name = (nc.partition_id_tensor.name
                      if nc.partition_id_tensor else None)

    in_names = []
    out_names = []
    out_avals = []
    for alloc in nc.m.functions[0].allocations:
        if not isinstance(alloc, mybir.MemoryLocationSet):
            continue
        assert alloc.memorylocations
        name = alloc.memorylocations[0].name
        if alloc.kind == "ExternalInput":
            if name != partition_name:
                in_names.append(name)
        elif alloc.kind == "ExternalOutput":
            assert alloc.tensor_shape is not None and alloc.dtype is not None
            out_names.append(name)
            shape = tuple(alloc.tensor_shape)
            dtype = mybir.dt.np(alloc.dtype)
            out_avals.append(jax.core.ShapedArray(shape, dtype))
    n_params = len(in_names)
    n_outs = len(out_avals)
    all_names = list(in_names) + list(out_names)
    if partition_name is not None:
        all_names.append(partition_name)
    donate = tuple(range(n_params, n_params + n_outs))

    def _body(*args):
        operands = list(args)
        if partition_name is not None:
            operands.append(bass2jax.partition_id_tensor())
        outs = bass2jax._bass_exec_p.bind(
            *operands,
            out_avals=tuple(out_avals),
            in_names=tuple(all_names),
            out_names=tuple(out_names),
            lowering_input_output_aliases=(),
            sim_require_finite=True,
            sim_require_nnan=True,
            nc=nc,
        )
        return tuple(outs)

    devices = jax.devices()[:N_CORES]
    assert len(devices) == N_CORES
    mesh = Mesh(np.asarray(devices), ("core",))
    pspec = PartitionSpec("core")
    sharding = NamedSharding(mesh, pspec)
    in_specs = (pspec,) * (n_params + n_outs)
    out_specs = (pspec,) * n_outs
    fn = jax.jit(
        shard_map(_body, mesh=mesh, in_specs=in_specs, out_specs=out_specs,
                  check_rep=False),
        donate_argnums=donate,
        keep_unused=True,
    )

    glob_out_shapes = [(N_CORES * a.shape[0],) + tuple(a.shape[1:])
                      for a in out_avals]
    glob_out_dtypes = [a.dtype for a in out_avals]

    def make_zeros():
        mk = jax.jit(
            lambda: tuple(jnp.zeros(s, d) for s, d in
                          zip(glob_out_shapes, glob_out_dtypes)),
            out_shardings=tuple(sharding for _ in glob_out_shapes),
        )
        return list(mk())

    ex = {
        "jax": jax,
        "fn": fn,
        "in_names": in_names,
        "out_names": out_names,
        "out_avals": out_avals,
        "sharding": sharding,
        "make_zeros": make_zeros,
    }
    _EXEC_CACHE[key] = ex
    return ex


def _stage(ex, in_maps):
    key = id(in_maps)
    hit = _STAGE_CACHE.get(key)
    if hit is not None and hit["pin"] is in_maps:
        return hit
    jax = ex["jax"]
    concat = [
        np.concatenate([np.asarray(m[name]) for m in in_maps], axis=0)
        for name in ex["in_names"]
    ]
    dev_inputs = [jax.device_put(a, ex["sharding"]) for a in concat]
    jax.block_until_ready(dev_inputs)
    staged = {
        "pin": in_maps,  # strong ref keeps id() stable
        "dev_inputs": dev_inputs,
        "donation": ex["make_zeros"](),
    }
    if len(_STAGE_CACHE) >= 2:  # bound device/host memory pinned by the cache
        _STAGE_CACHE.pop(next(iter(_STAGE_CACHE)))
    _STAGE_CACHE[key] = staged
    return staged


class _RunResult:
    """Per-run device outputs; host transfer happens lazily, once."""

    def __init__(self, ex, outs):
        self._ex = ex
        self._outs = outs
        self._host = None

    def host(self, name):
        if self._host is None:
            self._host = {
                n: np.asarray(a)
                for n, a in zip(self._ex["out_names"], self._outs)
            }
        return self._host[name]


class _CoreView:
    """numpy-convertible view of one core's slice of a global output."""

    def __init__(self, runres, name, core, core_shape):
        self._runres = runres
        self._name = name
        self._core = core
        self._core_shape = core_shape

    def __array__(self, dtype=None, copy=None):
        full = self._runres.host(self._name)
        arr = full.reshape((N_CORES,) + self._core_shape)[self._core]
        if dtype is not None:
            arr = arr.astype(dtype)
        return arr


def _execute(ex, staged):
    jax = ex["jax"]
    outs = list(ex["fn"](*staged["dev_inputs"], *staged["donation"]))
    jax.block_until_ready(outs)
    # recycle: this run's outputs become next run's donated buffers
    staged["donation"] = outs
    return _RunResult(ex, outs)




def run_on_cores(in_maps, reps=1):
    nc = _get_nc(reps)
    try:
        ex = _get_executor(nc)
        staged = _stage(ex, in_maps)
        rr = _execute(ex, staged)
        res = []
        for c in range(N_CORES):
            res.append({
                name: _CoreView(rr, name, c, tuple(aval.shape))
                for name, aval in zip(ex["out_names"], ex["out_avals"])
            })
        return res
    except Exception:
        from concourse.bass_utils import run_bass_kernel_spmd
        res = run_bass_kernel_spmd(nc, in_maps, core_ids=list(range(N_CORES)),
                                   trace=False)
        return res.results if hasattr(res, "results") else res


def kernel(Img: np.ndarray, DispField: np.ndarray) -> np.ndarray:
    in_maps = make_in_maps(Img, DispField)
    results = run_on_cores(in_maps)
    out = np.concatenate([np.asarray(r["out"]) for r in results], axis=0)
    return out.reshape(B_TOTAL, H, W, 1).astype(np.float32)


if __name__ == "__main__":
    rng = np.random.default_rng(0)
    Img = rng.random((B_TOTAL, H, W, 1), dtype=np.float32)
    Disp = rng.standard_normal((B_TOTAL, H, W, 2)).astype(np.float32)
    o = kernel(Img, Disp)
    print("out", o.shape, o.dtype, float(np.abs(o).mean()))
